# revision 1
# baseline (speedup 1.0000x reference)
"""Multi-head attention (16 heads, d_model=1024, head_dim=64) on 8 trn2 cores.

Sharding: core c handles batch b = c//2 and heads [8*(c%2), 8*(c%2)+8)
(data parallel over batch x tensor parallel over heads). Each core
computes its 8 heads' Q/K/V projections, attention, and a partial output
projection; the host sums the two partial projections per batch element
(the "all-reduce") and adds the output bias.

Device-side layout is feature-major ("transposed"): projections produce
Q^T/K^T [d, t] so that the attention matmuls contract along partitions.
Attention output is produced as AttnOut^T [f, t], which feeds the output
projection as the stationary operand without any transposes.

Q/K projections are computed per head-pair and pipelined into the
attention phase so the PE always has independent matmul work while the
scalar engine chews on exp (keeps the PE HAM clock warm).

All matmul inputs are bf16 (fp32 PSUM accumulation); softmax is
unnormalized exp (no max subtraction: energies are bounded ~|15| here)
with the row-sum computed by an extra ones-column in the attn@V matmul.
Measured end-to-end relative error vs the fp32 reference: ~9e-3.
"""

import numpy as np
import ml_dtypes

from concourse import bass, bacc, tile, mybir
from concourse.tile_rust import add_dep_helper
from concourse.bass_utils import run_bass_kernel_spmd

BF16 = ml_dtypes.bfloat16
dt = mybir.dt
AF = mybir.ActivationFunctionType

N_CORES = 8
T = 2048          # tokens per batch element
D = 1024          # model dim
FH = 512          # features (head dims) per core: 8 heads x 64
NH_LOC = 8        # heads per core
HD = 64           # head dim

_prog_cache = {}


def _build_program():
    nc = bacc.Bacc("TRN2", target_bir_lowering=False, debug=False,
                   num_devices=N_CORES)

    xT = nc.dram_tensor("xT", [D, T], dt.bfloat16, kind="ExternalInput").ap()
    wqT = nc.dram_tensor("wqT", [D, FH], dt.bfloat16, kind="ExternalInput").ap()
    wkT = nc.dram_tensor("wkT", [D, FH], dt.bfloat16, kind="ExternalInput").ap()
    wvT = nc.dram_tensor("wvT", [D, FH], dt.bfloat16, kind="ExternalInput").ap()
    bqT = nc.dram_tensor("bqT", [128, 4], dt.float32, kind="ExternalInput").ap()
    bkT = nc.dram_tensor("bkT", [128, 4], dt.float32, kind="ExternalInput").ap()
    bvs = nc.dram_tensor("bvs", [1, FH], dt.bfloat16, kind="ExternalInput").ap()
    wpT = nc.dram_tensor("wpT", [FH, D], dt.bfloat16, kind="ExternalInput").ap()
    ones = nc.dram_tensor("ones", [1, T], dt.bfloat16, kind="ExternalInput").ap()
    out = nc.dram_tensor("out", [T, D], dt.float32, kind="ExternalOutput").ap()

    with tile.TileContext(nc) as tc:
        _emit(tc, out, xT, wqT, wkT, wvT, bqT, bkT, bvs, wpT, ones)
    nc.compile()
    return nc


def _emit(tc, out, xT, wqT, wkT, wvT, bqT, bkT, bvs, wpT, ones):
    nc = tc.nc
    f32 = dt.float32
    bf16 = dt.bfloat16

    with (
        tc.tile_pool(name="sbp", bufs=1) as sbp,
        tc.tile_pool(name="qkv_sb", bufs=1) as qkv_sb,
        tc.tile_pool(name="pb_pool", bufs=3) as pb_pool,
        tc.tile_pool(name="rr_pool", bufs=2) as rr_pool,
        tc.tile_pool(name="bc_pool", bufs=2) as bc_pool,
        tc.tile_pool(name="ostage", bufs=2) as ostage,
        # PSUM: 4 banks for energies, 2 for attn@V accumulators, 2 shared
        # by V / per-pair Q,K accumulation / output projection.
        tc.tile_pool(name="ps_e", bufs=2, space="PSUM") as ps_e,
        tc.tile_pool(name="ps_av", bufs=2, space="PSUM") as ps_av,
        tc.tile_pool(name="ps_misc", bufs=2, space="PSUM") as ps_misc,
    ):
        # Input DMAs split across the two HW-DGE queues (SP + ACT), in
        # first-use order: Q/K weights and x feed the first matmuls; the
        # output-projection weight is not needed until the very end.
        ones_s = sbp.tile([1, T], bf16)
        nc.sync.dma_start(out=ones_s[:], in_=ones)
        bvs_s = sbp.tile([1, FH], bf16)
        nc.sync.dma_start(out=bvs_s[:], in_=bvs)
        bqT_s = sbp.tile([128, 4], f32)
        nc.sync.dma_start(out=bqT_s[:], in_=bqT)
        bkT_s = sbp.tile([128, 4], f32)
        nc.sync.dma_start(out=bkT_s[:], in_=bkT)

        wq_s = sbp.tile([128, 8, FH], bf16, tag="wq")
        nc.scalar.dma_start(out=wq_s[:], in_=wqT.rearrange("(m p) d -> p m d", p=128))
        wk_s = sbp.tile([128, 8, FH], bf16, tag="wk")
        nc.sync.dma_start(out=wk_s[:], in_=wkT.rearrange("(m p) d -> p m d", p=128))
        x_s = sbp.tile([128, 8, T], bf16)
        xr = xT.rearrange("(m p) t -> p m t", p=128)
        for m in range(8):
            eng = nc.sync if m % 2 == 0 else nc.scalar
            eng.dma_start(out=x_s[:, m, :], in_=xr[:, m, :])
        wv_s = sbp.tile([128, 8, FH], bf16, tag="wv")
        nc.scalar.dma_start(out=wv_s[:], in_=wvT.rearrange("(m p) d -> p m d", p=128))
        wp_s = sbp.tile([128, 4, D], bf16)
        nc.sync.dma_start(out=wp_s[:], in_=wpT.rearrange("(c p) o -> p c o", p=128))

        # QT/KT: [d-in-pair(128), head-pair(4), t]; V: [t-in-chunk(128),
        # t-chunk(16), head(8), 66] with col 64 = 1.0 (row-sum trick).
        QT_sb = qkv_sb.tile([128, 4, T], bf16)
        KT_sb = qkv_sb.tile([128, 4, T], bf16)
        V_sb = qkv_sb.tile([128, 16, NH_LOC, 66], bf16)
        nc.vector.memset(V_sb[:, :, :, 64:66], 1.0)
        # AttnOut^T: [f-in-chunk(128), f-chunk(4), t]
        AO_sb = qkv_sb.tile([128, 4, T], bf16)

        def emit_qk(hp):
            # Q^T/K^T rows for head-pair hp: out[d, t] = W[:, d].x[:, t] + b[d]
            dsl = slice(hp * 128, (hp + 1) * 128)
            for w_s, b_s, dst in ((wq_s, bqT_s, QT_sb), (wk_s, bkT_s, KT_sb)):
                for n in range(4):
                    ps = ps_misc.tile([128, 512], f32, tag="m", name="qk_ps")
                    for m in range(8):
                        nc.tensor.matmul(
                            ps[:], w_s[:, m, dsl],
                            x_s[:, m, n * 512:(n + 1) * 512],
                            start=(m == 0), stop=(m == 7))
                    nc.vector.tensor_scalar_add(
                        dst[:, hp, n * 512:(n + 1) * 512], ps[:],
                        b_s[:, hp:hp + 1])

        def emit_v_tile(t):
            # V (natural): out[t, d] = x[t, :].wvT[:, d] + bv[d]
            ps = ps_misc.tile([128, 512], f32, tag="m", name="v_ps")
            for m in range(8):
                nc.tensor.matmul(ps[:], x_s[:, m, t * 128:(t + 1) * 128],
                                 wv_s[:, m, :], start=(m == 0), stop=False)
            nc.tensor.matmul(ps[:], ones_s[:, t * 128:(t + 1) * 128],
                             bvs_s[:], start=False, stop=True)
            nc.vector.tensor_copy(
                V_sb[:, t, :, 0:64],
                ps[:].rearrange("p (h d) -> p h d", h=NH_LOC))

        def emit_qk_ntile(w_s, b_s, dst, hp, n, anchor=None):
            # one n-tile of a Q^T/K^T projection: an 8-matmul chain
            dsl = slice(hp * 128, (hp + 1) * 128)
            ps = ps_misc.tile([128, 512], f32, tag="m", name="qk_ps")
            for m in range(8):
                mm = nc.tensor.matmul(ps[:], w_s[:, m, dsl],
                                      x_s[:, m, n * 512:(n + 1) * 512],
                                      start=(m == 0), stop=(m == 7))
                if m == 0 and anchor is not None:
                    add_dep_helper(mm.ins, anchor.ins, sync=False,
                                   reason="filler pacing")
            nc.vector.tensor_scalar_add(
                dst[:, hp, n * 512:(n + 1) * 512], ps[:], b_s[:, hp:hp + 1])

        def emit_proj(t, anchor=None):
            # partial output projection (pre-bias) for token tile t
            tsl = slice(t * 128, (t + 1) * 128)
            st = ostage.tile([128, D], f32, tag="st")
            ps0 = ps_misc.tile([128, 512], f32, tag="m", name="pj0")
            ps1 = ps_misc.tile([128, 512], f32, tag="m", name="pj1")
            for fc in range(4):
                mm = nc.tensor.matmul(ps0[:], AO_sb[:, fc, tsl],
                                      wp_s[:, fc, 0:512],
                                      start=(fc == 0), stop=(fc == 3))
                if fc == 0 and anchor is not None:
                    add_dep_helper(mm.ins, anchor.ins, sync=False,
                                   reason="filler pacing")
                nc.tensor.matmul(ps1[:], AO_sb[:, fc, tsl],
                                 wp_s[:, fc, 512:1024],
                                 start=(fc == 0), stop=(fc == 3))
            nc.vector.tensor_copy(st[:, 0:512], ps0[:])
            nc.vector.tensor_copy(st[:, 512:1024], ps1[:])
            nc.sync.dma_start(out=out[tsl, :], in_=st[:])

        # ---- software-pipelined attention ----
        # Per head-unit u: emit u's energy matmuls + exp in 4 groups, and
        # interleave the PREVIOUS unit's attn@V blocks plus filler work
        # (next pair's Q/K n-tiles, output-projection tiles) into the exp
        # bubbles so the PE instruction queue never drains.
        units = [(hp, j, s) for hp in range(4) for j in range(4)
                 for s in range(2)]
        state = {}      # u -> (pb, av) live tiles
        fillers = []    # queue of zero-arg emitters

        def emit_e_group(u, g):
            # 2 key-chunks per group: two 2-bank PSUM tiles ping-pong so the
            # energy matmuls and exp pipeline instead of alternating
            hp, j, s = u
            psl = slice(64 * s, 64 * s + 64)
            qsl = slice(j * 512, (j + 1) * 512)
            pb = state[u][0]
            e2 = ps_e.tile([128, 2, 512], f32, tag="e")
            for i in range(2):
                kc = 2 * g + i
                ksl = slice(kc * 128, (kc + 1) * 128)
                nc.tensor.matmul(e2[:, i, :], KT_sb[psl, hp, ksl],
                                 QT_sb[psl, hp, qsl], start=True, stop=True)
            return nc.scalar.activation(pb[:, 2 * g:2 * g + 2, :], e2[:],
                                        AF.Exp)

        def emit_av_block(u, kcs):
            # attn@V accumulation matmuls (V col 64 is ones -> row sums)
            hp, j, s = u
            h = 2 * hp + s
            pb, av = state[u][0], state[u][1]
            if av is None:
                av = ps_av.tile([128, 512], f32, tag="av")
                state[u] = (pb, av, None, None)
            for kc in kcs:
                nc.tensor.matmul(av[0:65, :], V_sb[:, kc, h, 0:65],
                                 pb[:, kc, :],
                                 start=(kc == 0), stop=(kc == 15))

        def emit_norm_a(u):
            # softmax normalization part 1 (DVE only): copy the accumulator
            # rows to SBUF and take the reciprocal of the row sums. Runs
            # right after the attn@V accumulation so the av PSUM bank frees
            # early. The reciprocal is slow (~3.2us) - part 2 consumes it
            # a full unit later so its latency never stalls the PE.
            av = state[u][1]
            avd = bc_pool.tile([64, 512], bf16, tag="avd", bufs=4)
            nc.vector.tensor_copy(avd[:], av[0:64, :])
            rr = rr_pool.tile([1, 512], f32, tag="rr", bufs=4)
            nc.vector.reciprocal(rr[:], av[64:65, :])
            rrb = rr_pool.tile([1, 512], bf16, tag="rrb", bufs=4)
            nc.vector.tensor_copy(rrb[:], rr[:])
            state[u] = (None, None, avd, rrb)

        def emit_norm_b(u):
            # part 2: broadcast 1/rowsum across the 64 head-dim partitions
            # via a K=1 matmul, then scale into AttnOut^T
            hp, j, s = u
            psl = slice(64 * s, 64 * s + 64)
            qsl = slice(j * 512, (j + 1) * 512)
            avd, rrb = state[u][2], state[u][3]
            bcp = ps_misc.tile([64, 512], f32, tag="m", name="bcp")
            nc.tensor.matmul(bcp[:], ones_s[0:1, 0:64], rrb[:],
                             start=True, stop=True)
            nc.vector.tensor_mul(AO_sb[psl, hp, qsl], avd[:], bcp[:])
            del state[u]

        def pop_filler(anchor=None):
            if fillers:
                fillers.pop(0)(anchor)

        emit_qk(0)
        prev = None
        prev2 = None
        for ui, u in enumerate(units):
            hp, j, s = u
            if j == 0 and s == 0 and hp < 3:
                # queue next pair's Q/K projection tiles as PE filler
                for w_s, b_s, dst in ((wq_s, bqT_s, QT_sb),
                                      (wk_s, bkT_s, KT_sb)):
                    for n in range(4):
                        fillers.append(
                            lambda a, w=w_s, b=b_s, d=dst, p=hp + 1, nn=n:
                            emit_qk_ntile(w, b, d, p, nn, anchor=a))
            state[u] = (pb_pool.tile([128, 16, 512], bf16, tag="pb",
                                     name="pb"), None, None, None)
            # Interleave: E groups of u, attn@V chains of prev, and one or
            # two filler tiles, paced so every unit's PE work slightly
            # exceeds its exp time (keeps the PE dense and the HAM clock
            # warm).
            if prev is not None:
                emit_av_block(prev, range(0, 8))
            exp0 = emit_e_group(u, 0)
            if ui == 0:
                for t in range(0, 6):
                    emit_v_tile(t)
            emit_e_group(u, 1)
            if ui >= 2 and fillers:
                pop_filler(exp0)
            elif hp == 3 and j <= 1:
                # seam between the last qk filler and the first projection
                # filler: no independent work exists, so re-emit a Q/K
                # n-tile (idempotent) to keep the PE dense and warm
                emit_qk_ntile(wk_s, bkT_s, KT_sb, 3, 2 * j + s, anchor=exp0)
            emit_e_group(u, 2)
            if ui == 0:
                for t in range(6, 11):
                    emit_v_tile(t)
            emit_e_group(u, 3)
            if prev is not None:
                emit_av_block(prev, range(8, 16))
            exp4 = emit_e_group(u, 4)
            if ui == 0:
                for t in range(11, 16):
                    emit_v_tile(t)
            emit_e_group(u, 5)
            if ui >= 2 and (hp == 3 or j == 3):
                pop_filler(exp4)
            if hp == 3 and j >= 2:
                pop_filler(exp4)
            if prev is not None:
                emit_norm_a(prev)
            if prev2 is not None:
                # norm_b two units after the accumulation: the reciprocal's
                # ~3.2us latency is fully covered before the broadcast
                # matmul enters the PE stream
                emit_norm_b(prev2)
                php, pj, ps_ = prev2
                if php == 3 and ps_ == 1:
                    # all heads done for token range pj -> queue projection
                    for tt in range(4):
                        fillers.append(
                            lambda a, t=4 * pj + tt: emit_proj(t, anchor=a))
            emit_e_group(u, 6)
            emit_e_group(u, 7)
            prev2 = prev
            prev = u
        # pipeline tail
        emit_av_block(prev, range(0, 8))
        emit_av_block(prev, range(8, 16))
        emit_norm_b(prev2)
        emit_norm_a(prev)
        emit_norm_b(prev)
        for tt in range(4):
            fillers.append(lambda a, t=12 + tt: emit_proj(t, anchor=a))
        while fillers:
            pop_filler()


def get_program():
    if "nc" not in _prog_cache:
        _prog_cache["nc"] = _build_program()
    return _prog_cache["nc"]


def make_in_maps(inputs):
    x = np.asarray(inputs["x"], dtype=np.float32)
    Wq = np.asarray(inputs["Wq"], dtype=np.float32)
    bq = np.asarray(inputs["bq"], dtype=np.float32)
    Wk = np.asarray(inputs["Wk"], dtype=np.float32)
    bk = np.asarray(inputs["bk"], dtype=np.float32)
    Wv = np.asarray(inputs["Wv"], dtype=np.float32)
    bv = np.asarray(inputs["bv"], dtype=np.float32)
    Wp = np.asarray(inputs["Wp"], dtype=np.float32)

    ones_h = np.ones((1, T), dtype=BF16)
    in_maps = []
    for c in range(N_CORES):
        b, half = divmod(c, 2)
        fs = slice(half * FH, half * FH + FH)
        in_maps.append({
            "xT": np.ascontiguousarray(x[b].T).astype(BF16),
            "wqT": np.ascontiguousarray(Wq[fs].T).astype(BF16),
            "wkT": np.ascontiguousarray(Wk[fs].T).astype(BF16),
            "wvT": np.ascontiguousarray(Wv[fs].T).astype(BF16),
            "bqT": np.ascontiguousarray(bq[fs].reshape(4, 128).T),
            "bkT": np.ascontiguousarray(bk[fs].reshape(4, 128).T),
            "bvs": bv[fs].astype(BF16).reshape(1, FH),
            "wpT": np.ascontiguousarray(Wp[:, fs].T).astype(BF16),
            "ones": ones_h,
        })
    return in_maps


def gather_output(results, bp):
    bp = np.asarray(bp, dtype=np.float32)
    return np.stack([
        results[2 * b]["out"] + results[2 * b + 1]["out"] + bp[None, :]
        for b in range(4)
    ]).astype(np.float32)


def kernel(**inputs):
    nc = get_program()
    in_maps = make_in_maps(inputs)
    res = run_bass_kernel_spmd(nc, in_maps, list(range(N_CORES))).results
    return gather_output(res, inputs["bp"])



# revision 15
# speedup vs baseline: 8156.0076x; 8156.0076x over previous
"""Multi-head attention (16 heads, d_model=1024, head_dim=64) on 8 trn2 cores.

Sharding: core c handles batch b = c//2 and heads [8*(c%2), 8*(c%2)+8)
(data parallel over batch x tensor parallel over heads). Each core
computes its 8 heads' Q/K/V projections, attention, and a partial output
projection; the host sums the two partial projections per batch element
(the "all-reduce") and adds the output bias.

Device-side layout is feature-major ("transposed"): projections produce
Q^T/K^T [d, t] so that the attention matmuls contract along partitions.
Attention output is produced as AttnOut^T [f, t], which feeds the output
projection as the stationary operand without any transposes.

Schedule: query-block-outer unit order (j, hp, s). All K^T tiles, V
tiles and the first Q^T tiles are produced by a deadline-tagged filler
queue that the attention units drain at ~2-matmul granularity, paced so
every exp's PSUM ping-pong buffer is free by the time the PE reaches the
matmul that reuses it. Output-projection tiles enter the same queue as
soon as their query block's heads are normalized, which spreads the
final projection across the whole kernel instead of piling it into the
tail (long dense matmul bursts there trip the PE power throttle).
Input DMAs are split fine-grained across three DGE queues in first-use
order so the PE starts within a few us of launch.

All matmul inputs are bf16 (fp32 PSUM accumulation); softmax is
unnormalized exp (no max subtraction: energies are bounded ~|15| here)
with the row-sum computed by an extra ones-column in the attn@V matmul.
Row-sum reciprocals use the fast approximate DVE reciprocal (~18
correct bits, plenty ahead of a bf16 multiply).
Measured end-to-end relative error vs the fp32 reference: ~9e-3.
"""

import numpy as np
import ml_dtypes

from concourse import bass, bacc, tile, mybir
from concourse.tile_rust import add_dep_helper
from concourse.bass_utils import run_bass_kernel_spmd

BF16 = ml_dtypes.bfloat16
dt = mybir.dt
AF = mybir.ActivationFunctionType

N_CORES = 8
T = 2048          # tokens per batch element
D = 1024          # model dim
FH = 512          # features (head dims) per core: 8 heads x 64
NH_LOC = 8        # heads per core
HD = 64           # head dim

# PE rows emitted per exp slot: one exp covers 1024 elements/partition
# (~1094ns incl. access latency); 2625 rows ~= the same span at 2.4GHz.
SLOT_ROWS = 2625

_prog_cache = {}


def _build_program():
    nc = bacc.Bacc("TRN2", target_bir_lowering=False, debug=False,
                   num_devices=N_CORES)

    xT = nc.dram_tensor("xT", [D, T], dt.bfloat16, kind="ExternalInput").ap()
    wqT = nc.dram_tensor("wqT", [D, FH], dt.bfloat16, kind="ExternalInput").ap()
    wkT = nc.dram_tensor("wkT", [D, FH], dt.bfloat16, kind="ExternalInput").ap()
    wvT = nc.dram_tensor("wvT", [D, FH], dt.bfloat16, kind="ExternalInput").ap()
    bqT = nc.dram_tensor("bqT", [128, 4], dt.float32, kind="ExternalInput").ap()
    wpT = nc.dram_tensor("wpT", [FH, D], dt.bfloat16, kind="ExternalInput").ap()
    ones = nc.dram_tensor("ones", [1, T], dt.bfloat16, kind="ExternalInput").ap()
    out = nc.dram_tensor("out", [T, D], dt.float32, kind="ExternalOutput").ap()

    with tile.TileContext(nc) as tc:
        _emit(tc, out, xT, wqT, wkT, wvT, bqT, wpT, ones)
    nc.compile()
    return nc


def _emit(tc, out, xT, wqT, wkT, wvT, bqT, wpT, ones):
    nc = tc.nc
    f32 = dt.float32
    bf16 = dt.bfloat16

    with (
        tc.tile_pool(name="sbp", bufs=1) as sbp,
        tc.tile_pool(name="qkv_sb", bufs=1) as qkv_sb,
        tc.tile_pool(name="pb_pool", bufs=3) as pb_pool,
        tc.tile_pool(name="rr_pool", bufs=2) as rr_pool,
        tc.tile_pool(name="bc_pool", bufs=2) as bc_pool,
        tc.tile_pool(name="ostage", bufs=2) as ostage,
        # PSUM: 4 banks for energies, 2 for attn@V accumulators, 2 shared
        # by the filler chains (Q/K/V/proj tiles) and norm broadcasts.
        tc.tile_pool(name="ps_e", bufs=2, space="PSUM") as ps_e,
        tc.tile_pool(name="ps_av", bufs=2, space="PSUM") as ps_av,
        tc.tile_pool(name="ps_misc", bufs=2, space="PSUM") as ps_misc,
    ):
        # Input DMAs: both HW DGE queues, first-use order, fine-grained so
        # the first K-projection matmul can start after ~1.5MB has landed.
        #  - ACT queue: weights (wk/wq pair-halves first).
        #  - SP queue: x in 16 [128, 2, 512] pieces, token-block-major,
        #    with the tiny tensors (bq, ones) slotted between.
        # The K bias is dropped entirely: softmax is invariant to the
        # per-query-row constant (q+bq).bk it contributes. The V bias is
        # folded into the output bias on the host (attention rows sum to
        # 1, so attn@(V+1.bv).Wp^T = attn@V.Wp^T + Wp@bv).
        wk_s = sbp.tile([128, 8, FH], bf16, tag="wk")
        wkr = wkT.rearrange("(m p) d -> p m d", p=128)
        wq_s = sbp.tile([128, 8, FH], bf16, tag="wq")
        wqr = wqT.rearrange("(m p) d -> p m d", p=128)
        for h2 in range(2):
            fsl = slice(h2 * 256, (h2 + 1) * 256)
            nc.scalar.dma_start(out=wk_s[:, :, fsl], in_=wkr[:, :, fsl])
        for h2 in range(2):
            fsl = slice(h2 * 256, (h2 + 1) * 256)
            nc.scalar.dma_start(out=wq_s[:, :, fsl], in_=wqr[:, :, fsl])
        wv_s = sbp.tile([128, 8, FH], bf16, tag="wv")
        nc.scalar.dma_start(out=wv_s[:], in_=wvT.rearrange("(m p) d -> p m d", p=128))
        wp_s = sbp.tile([128, 4, D], bf16)
        nc.scalar.dma_start(out=wp_s[:], in_=wpT.rearrange("(c p) o -> p c o", p=128))

        x_s = sbp.tile([128, 8, T], bf16)
        xr = xT.rearrange("(m p) t -> p m t", p=128)
        ones_s = sbp.tile([1, T], bf16)
        bqT_s = sbp.tile([128, 4], f32)
        for n in range(4):
            nsl = slice(n * 512, (n + 1) * 512)
            for mg in range(4):
                msl = slice(2 * mg, 2 * mg + 2)
                nc.sync.dma_start(out=x_s[:, msl, nsl], in_=xr[:, msl, nsl])
            if n == 0:
                nc.sync.dma_start(out=bqT_s[:], in_=bqT)
                nc.sync.dma_start(out=ones_s[:], in_=ones)

        # QT/KT: [d-in-pair(128), head-pair(4), t]; V: [t-in-chunk(128),
        # t-chunk(16), head(8), 66] with col 64 = 1.0 (row-sum trick).
        QT_sb = qkv_sb.tile([128, 4, T], bf16)
        KT_sb = qkv_sb.tile([128, 4, T], bf16)
        V_sb = qkv_sb.tile([128, 16, NH_LOC, 66], bf16)
        nc.vector.memset(V_sb[:, :, :, 64:66], 1.0)
        # AttnOut^T: [f-in-chunk(128), f-chunk(4), t]
        AO_sb = qkv_sb.tile([128, 4, T], bf16)

        # ---- pacing anchor: pin filler matmuls at/after the latest exp
        # in the static schedule (ordering-only edge, no runtime sem).
        cur_anchor = [None]

        def A(mm):
            if cur_anchor[0] is not None:
                add_dep_helper(mm.ins, cur_anchor[0].ins, sync=False,
                               reason="filler pacing")

        # ---- filler generators: yield PE rows after every ~2 matmuls ----
        def gen_qk_tile(w_s, b_s, dst, hp, n):
            # one [128 feat, 512 tok] tile of a Q^T/K^T projection
            dsl = slice(hp * 128, (hp + 1) * 128)
            nsl = slice(n * 512, (n + 1) * 512)
            ps = ps_misc.tile([128, 512], f32, tag="m", name="qk_ps")
            for m in range(8):
                mm = nc.tensor.matmul(ps[:], w_s[:, m, dsl], x_s[:, m, nsl],
                                      start=(m == 0), stop=(m == 7))
                if m % 2 == 0:
                    A(mm)
                if m % 2 == 1 and m < 7:
                    yield 1024
            if b_s is None:
                nc.vector.tensor_copy(dst[:, hp, nsl], ps[:])
            else:
                nc.vector.tensor_scalar_add(dst[:, hp, nsl], ps[:],
                                            b_s[:, hp:hp + 1])
            yield 1024

        def gen_v_tile(t):
            # V (natural, bias-free): out[t, d] = x[t, :].wvT[:, d]
            tsl = slice(t * 128, (t + 1) * 128)
            ps = ps_misc.tile([128, 512], f32, tag="m", name="v_ps")
            for m in range(8):
                mm = nc.tensor.matmul(ps[:], x_s[:, m, tsl], wv_s[:, m, :],
                                      start=(m == 0), stop=(m == 7))
                if m % 2 == 0:
                    A(mm)
                if m % 2 == 1 and m < 7:
                    yield 1024
            nc.vector.tensor_copy(
                V_sb[:, t, :, 0:64],
                ps[:].rearrange("p (h d) -> p h d", h=NH_LOC))
            yield 1024

        def gen_proj(t):
            # partial output projection (pre-bias) for token tile t
            tsl = slice(t * 128, (t + 1) * 128)
            st = ostage.tile([128, D], f32, tag="st")
            for half in range(2):
                osl = slice(half * 512, (half + 1) * 512)
                ps = ps_misc.tile([128, 512], f32, tag="m", name="pj")
                for fc in range(4):
                    mm = nc.tensor.matmul(ps[:], AO_sb[:, fc, tsl],
                                          wp_s[:, fc, osl],
                                          start=(fc == 0), stop=(fc == 3))
                    if fc == 0:
                        A(mm)
                    if fc == 1:
                        yield 1024
                nc.vector.tensor_copy(st[:, osl], ps[:])
                if half == 0:
                    yield 1024
            nc.sync.dma_start(out=out[tsl, :], in_=st[:])
            yield 1024

        # ---- deadline-tagged filler queue ----
        # Entries are (deadline_unit_idx, generator); pushed in deadline
        # order, drained head-first. force_drain() runs everything the
        # upcoming unit depends on; paced pops spread the rest.
        fillers = []

        def push(dl, gen):
            fillers.append((dl, gen))

        def pop_chunk():
            while fillers:
                try:
                    return next(fillers[0][1])
                except StopIteration:
                    fillers.pop(0)
            return 0

        def force_drain(ui):
            while fillers and fillers[0][0] <= ui:
                try:
                    next(fillers[0][1])
                except StopIteration:
                    fillers.pop(0)

        for n in range(4):
            push(0, gen_qk_tile(wk_s, None, KT_sb, 0, n))
        push(0, gen_qk_tile(wq_s, bqT_s, QT_sb, 0, 0))
        for t in range(16):
            push(1, gen_v_tile(t))
        for hp in range(1, 4):
            for n in range(4):
                push(2 * hp, gen_qk_tile(wk_s, None, KT_sb, hp, n))
            push(2 * hp, gen_qk_tile(wq_s, bqT_s, QT_sb, hp, 0))
        for j in range(1, 4):
            for hp in range(4):
                push(8 * j + 2 * hp, gen_qk_tile(wq_s, bqT_s, QT_sb, hp, j))

        # ---- softmax normalization, staggered to hide latencies ----
        pbs = {}      # unit -> exp(probs) tile
        avs = {}      # unit -> attn@V PSUM accumulator
        norm_st = {}  # unit -> (avd, rrb)

        def emit_norm_a(u):
            # part 1 (DVE only): copy the accumulator rows + row sums to
            # SBUF right after the attn@V accumulation (frees the av PSUM
            # bank before the next unit allocates its accumulator), then
            # take the reciprocal of the SBUF copy of the sums.
            av = avs.pop(u)
            avd = bc_pool.tile([64, 512], bf16, tag="avd", bufs=3)
            nc.vector.tensor_copy(avd[:], av[0:64, :])
            sums = rr_pool.tile([1, 512], f32, tag="sums", bufs=2)
            nc.vector.tensor_copy(sums[:], av[64:65, :])
            rr = rr_pool.tile([1, 512], f32, tag="rr", bufs=2)
            nc.vector.reciprocal(rr[:], sums[:])
            rrb = rr_pool.tile([1, 512], bf16, tag="rrb", bufs=2)
            nc.vector.tensor_copy(rrb[:], rr[:])
            norm_st[u] = (avd, rrb)
            del pbs[u]

        def emit_norm_b(u):
            # part 2: broadcast 1/rowsum across the 64 head-dim partitions
            # via a K=1 matmul, then scale into AttnOut^T. When this was
            # the last head of a query block, its projection tiles become
            # available as filler.
            j, hp, s = u
            psl = slice(64 * s, 64 * s + 64)
            qsl = slice(j * 512, (j + 1) * 512)
            avd, rrb = norm_st.pop(u)
            bcp = ps_misc.tile([64, 512], f32, tag="m", name="bcp")
            mm = nc.tensor.matmul(bcp[:], ones_s[0:1, 0:64], rrb[:],
                                  start=True, stop=True)
            A(mm)
            nc.vector.tensor_mul(AO_sb[psl, hp, qsl], avd[:], bcp[:])
            if hp == 3 and s == 1:
                for tt in range(4):
                    push(32, gen_proj(4 * j + tt))

        # ---- attention units, query-block outer ----
        units = [(j, hp, s) for j in range(4) for hp in range(4)
                 for s in range(2)]
        prev = None
        prev2 = None
        for ui, u in enumerate(units):
            force_drain(ui)
            j, hp, s = u
            psl = slice(64 * s, 64 * s + 64)
            qsl = slice(j * 512, (j + 1) * 512)
            pb = pb_pool.tile([128, 16, 512], bf16, tag="pb", name="pb")
            pbs[u] = pb
            if prev is not None:
                avs[prev] = ps_av.tile([128, 512], f32, tag="av", name="av")
                ph = 2 * prev[1] + prev[2]
                ppb = pbs[prev]
                pav = avs[prev]
            rows = 0
            target = 0
            for g in range(8):
                if prev is not None:
                    # prev unit's attn@V (V col 64 is ones -> row sums)
                    for kc in (2 * g, 2 * g + 1):
                        mm = nc.tensor.matmul(pav[0:65, :],
                                              V_sb[:, kc, ph, 0:65],
                                              ppb[:, kc, :],
                                              start=(kc == 0),
                                              stop=(kc == 15))
                        if kc % 2 == 0:
                            A(mm)
                    rows += 1024
                    if g == 7:
                        # norm_a right after the accumulation closes so
                        # the av bank frees before the next unit needs it
                        emit_norm_a(prev)
                e2 = ps_e.tile([128, 2, 512], f32, tag="e")
                for i in range(2):
                    kc = 2 * g + i
                    nc.tensor.matmul(e2[:, i, :],
                                     KT_sb[psl, hp, kc * 128:(kc + 1) * 128],
                                     QT_sb[psl, hp, qsl],
                                     start=True, stop=True)
                rows += 1024
                cur_anchor[0] = nc.scalar.activation(
                    pb[:, 2 * g:2 * g + 2, :], e2[:], AF.Exp)
                if g == 4 and prev2 is not None:
                    emit_norm_b(prev2)
                    rows += 512
                target += SLOT_ROWS
                while rows < target:
                    r = pop_chunk()
                    if r == 0:
                        break
                    rows += r
            prev2 = prev
            prev = u

        # ---- pipeline tail ----
        avs[prev] = ps_av.tile([128, 512], f32, tag="av", name="av")
        ph = 2 * prev[1] + prev[2]
        ppb = pbs[prev]
        pav = avs[prev]
        for kc in range(16):
            nc.tensor.matmul(pav[0:65, :], V_sb[:, kc, ph, 0:65],
                             ppb[:, kc, :], start=(kc == 0), stop=(kc == 15))
            if kc == 7:
                emit_norm_b(prev2)
        emit_norm_a(prev)
        while fillers:       # remaining non-final projection tiles
            if pop_chunk() == 0:
                break
        emit_norm_b(prev)    # pushes the last query block's projections
        while fillers:
            if pop_chunk() == 0:
                break


def get_program():
    if "nc" not in _prog_cache:
        _prog_cache["nc"] = _build_program()
    return _prog_cache["nc"]


def make_in_maps(inputs):
    x = np.asarray(inputs["x"], dtype=np.float32)
    Wq = np.asarray(inputs["Wq"], dtype=np.float32)
    bq = np.asarray(inputs["bq"], dtype=np.float32)
    Wk = np.asarray(inputs["Wk"], dtype=np.float32)
    bk = np.asarray(inputs["bk"], dtype=np.float32)
    Wv = np.asarray(inputs["Wv"], dtype=np.float32)
    bv = np.asarray(inputs["bv"], dtype=np.float32)
    Wp = np.asarray(inputs["Wp"], dtype=np.float32)

    ones_h = np.ones((1, T), dtype=BF16)
    in_maps = []
    for c in range(N_CORES):
        b, half = divmod(c, 2)
        fs = slice(half * FH, half * FH + FH)
        in_maps.append({
            "xT": np.ascontiguousarray(x[b].T).astype(BF16),
            "wqT": np.ascontiguousarray(Wq[fs].T).astype(BF16),
            "wkT": np.ascontiguousarray(Wk[fs].T).astype(BF16),
            "wvT": np.ascontiguousarray(Wv[fs].T).astype(BF16),
            "bqT": np.ascontiguousarray(bq[fs].reshape(4, 128).T),
            "wpT": np.ascontiguousarray(Wp[:, fs].T).astype(BF16),
            "ones": ones_h,
        })
    return in_maps


def gather_output(results, bp_eff):
    return np.stack([
        results[2 * b]["out"] + results[2 * b + 1]["out"] + bp_eff[None, :]
        for b in range(4)
    ]).astype(np.float32)


def kernel(**inputs):
    nc = get_program()
    in_maps = make_in_maps(inputs)
    # attention rows sum to 1, so the V bias reaches the output as Wp@bv
    bp_eff = (np.asarray(inputs["bp"], np.float32)
              + np.asarray(inputs["Wp"], np.float32)
              @ np.asarray(inputs["bv"], np.float32))
    res = run_bass_kernel_spmd(nc, in_maps, list(range(N_CORES))).results
    return gather_output(res, bp_eff)
